# revision 2
# baseline (speedup 1.0000x reference)
"""Bass/Trainium2 attention kernel for nn_AttentionModule_39462159515861.

HW-calibrated redesign (measured per-instruction rates on this axon trn2):
  - K=64 fp8 matmul: ~345ns; K=128 fp8: ~187ns; bf16/f32r K=128: ~230-240ns;
    fp8 DoubleRow (2x K=128 per instr): ~238ns => 119ns per K=128 chunk.
  - exp [128,512] PSUM->SBUF in-stream: ~520ns (1 elem/lane/cycle + ~90ns).

Changes vs the baseline:
  - QK contracts K=128: per-head q/k fp8 tiles zero-padded on the unused
    partition half (even head: data on 0:64, odd head: on 64:128).
    Zero rows contribute nothing -> exact scores, full-K PE rate.
  - QKV projection (stage 1) runs in fp8 with DoubleRow (pairs of 128-row
    contraction chunks per instruction): 192 instead of 384 matmuls.
  - probs and v are fp8; PV uses DoubleRow over mt-chunk pairs: 256 MMs.
  - Output projection (stage 3) uses fp8 cat/w_proj with DoubleRow: 64 MMs.
  - Scores pipeline: 4-deep [128,512] PSUM tiles; exp is [128,512].
  - Output partials are bf16 (host sums in f32).
Sharding: 8 cores = (batch b in 0..3) x (head-group g in 0..1).
Accumulations (PSUM), softmax denominator, and normalization stay f32.
"""

import sys
import time

sys.path.insert(0, "/opt/trn_rl_repo")

import numpy as np

import concourse.bass as bass
import concourse.mybir as mybir
from concourse import bacc
from concourse.tile import TileContext

DIM = 1024
HEADS = 16
HD = 64
B = 4
N = 2048
GH = 8           # heads per core
GI = GH * HD     # 512 inner dims per core
P = 128
FP = mybir.dt.float32
F8 = mybir.dt.float8e4
BF = mybir.dt.bfloat16
SCALE = HD ** -0.5
DR = mybir.MatmulPerfMode.DoubleRow
FPR = mybir.dt.float32r

S1Q_FP8 = True   # q,k projections via fp8 DoubleRow (scores path)
PV_FP8 = True    # probs+v fp8 with DoubleRow PV (author-measured ~1.5e-2)


def _mm_cast(ap):
    return ap.bitcast(FPR)

NC8 = DIM // P       # 8 c-chunks
NCP = NC8 // 2       # 4 c-chunk pairs (DoubleRow)
NT = N // P          # 16 token tiles
NTP = NT // 2        # 8 token-tile pairs
N4 = N // 512        # 4 n-chunks of 512
VW = HD + 1          # 65: v columns + ones column
VROW = GH * VW       # 520 used v columns
VPAD = 528           # padded pair stride (multiple of 16 bytes)
SW = 512             # scores tile width (q tokens per exp tile)
NQS = N // SW        # 4 q-slices per head


def build_nc(reps=1, only=None, fill=8000):
    nc = bacc.Bacc("TRN2", target_bir_lowering=False, debug=False, num_devices=8)

    xT = nc.dram_tensor("xT", [DIM, N], FP, kind="ExternalInput").ap()
    wqkvT = nc.dram_tensor("wqkvT", [DIM, 3 * GI], FP, kind="ExternalInput").ap()
    bqk = nc.dram_tensor("bqk", [2 * GI], FP, kind="ExternalInput").ap()
    bv = nc.dram_tensor("bv", [GI], FP, kind="ExternalInput").ap()
    wpT = nc.dram_tensor("wpT", [GI, DIM], FP, kind="ExternalInput").ap()
    bph = nc.dram_tensor("bph", [DIM], FP, kind="ExternalInput").ap()
    part = nc.dram_tensor("part", [N, DIM], BF, kind="ExternalOutput").ap()

    with TileContext(nc) as tc, nc.allow_low_precision(reason="fp8/bf16 attention"):
        with (
            tc.tile_pool(name="persist", bufs=1) as persist,
            tc.tile_pool(name="small", bufs=1) as small,
            tc.tile_pool(name="wqf_pool", bufs=1) as wqf_pool,
            tc.tile_pool(name="wq_pool", bufs=1) as wq_pool,
            tc.tile_pool(name="wqv_pool", bufs=1) as wqv_pool,
            tc.tile_pool(name="x_pool", bufs=9) as x_pool,
            tc.tile_pool(name="x8_pool", bufs=5) as x8_pool,
            tc.tile_pool(name="probs", bufs=3) as probs_pool,
            tc.tile_pool(name="zrpool", bufs=1) as zr_pool,
            tc.tile_pool(name="zpool", bufs=2) as z_pool,
            tc.tile_pool(name="wpf_pool", bufs=1) as wpf_pool,
            tc.tile_pool(name="wpb_pool", bufs=1) as wpb_pool,
            tc.tile_pool(name="outp", bufs=2) as outp,
            tc.tile_pool(name="psA", bufs=2, space="PSUM") as psA,
            tc.tile_pool(name="ps2", bufs=2, space="PSUM") as ps2,
            tc.tile_pool(name="pso", bufs=2, space="PSUM") as pso,
        ):
            # Per-head q/k fp8 tiles, zero-padded on the unused partition
            # half (even head: data 0:64, odd head: data 64:128).
            q8h = [[persist.tile([P, N], F8, name=f"q8_{par}_{h}") for h in range(GH)]
                   for par in range(2)]
            k8h = [[persist.tile([P, N], F8, name=f"k8_{par}_{h}") for h in range(GH)]
                   for par in range(2)]
            if PV_FP8:
                # v in fp8, mt-chunk pairs for DoubleRow PV: [128, 2, VPAD].
                v8p = [[persist.tile([P, 2, VPAD], F8, name=f"v{par}_{i}")
                        for i in range(NTP)] for par in range(2)]
            else:
                v8p = [[persist.tile([P, GH * VW], BF, name=f"v{par}_{i}")
                        for i in range(NT)] for par in range(2)]
            # cat (normalized attention out) bf16, 4 ic-chunk tiles.
            cat_sb = [[persist.tile([P, N], BF, name=f"cat{par}_{i}") for i in range(4)]
                      for par in range(2)]

            for par in range(2):
                for h in range(GH):
                    nc.vector.memset(q8h[par][h], 0.0)
                    nc.vector.memset(k8h[par][h], 0.0)

            bqk_sb = small.tile([P, 8], FP, name="bqk_sb")
            nc.sync.dma_start(out=bqk_sb, in_=bqk.rearrange("(jt p) -> p jt", p=P))
            bv_bc = small.tile([P, GI], FP, name="bv_bc")
            nc.sync.dma_start(
                out=bv_bc, in_=bv.rearrange("(one j) -> one j", one=1).partition_broadcast(P)
            )
            bp_bc = small.tile([P, DIM], FP, name="bp_bc")
            nc.sync.dma_start(
                out=bp_bc, in_=bph.rearrange("(one j) -> one j", one=1).partition_broadcast(P)
            )
            # ones columns of v_aug (written once; stage 1 only writes 0:HD)
            ones_f32 = small.tile([P, GH], FP, name="ones_f32")
            nc.vector.memset(ones_f32, 1.0)
            if PV_FP8:
                for par in range(2):
                    for i in range(NTP):
                        for j in range(2):
                            vv = v8p[par][i][:, j, 0:VROW].rearrange(
                                "p (h w) -> p h w", w=VW
                            )
                            nc.vector.tensor_copy(
                                vv[:, :, HD : HD + 1],
                                ones_f32.rearrange("p (h w) -> p h w", w=1),
                            )
            else:
                for par in range(2):
                    for mt in range(NT):
                        vv = v8p[par][mt].rearrange("p (h w) -> p h w", w=VW)
                        nc.vector.tensor_copy(
                            vv[:, :, HD : HD + 1],
                            ones_f32.rearrange("p (h w) -> p h w", w=1),
                        )

            st = dict(
                nc=nc, xT=xT, wqkvT=wqkvT, wpT=wpT, part=part,
                q8h=q8h, k8h=k8h, v8p=v8p, cat_sb=cat_sb,
                bqk_sb=bqk_sb, bv_bc=bv_bc, bp_bc=bp_bc,
                wqf_pool=wqf_pool, wq_pool=wq_pool, wqv_pool=wqv_pool,
                x_pool=x_pool, x8_pool=x8_pool, probs_pool=probs_pool,
                z_pool=z_pool, zr_pool=zr_pool, wpf_pool=wpf_pool, wpb_pool=wpb_pool,
                outp=outp, psA=psA, ps2=ps2, pso=pso,
                wq_sb={}, wp_sb={}, wqv_sb={},
            )

            # Stage 2 is one flattened stream per rep (no per-block
            # boundaries); stage3(i-1) and stage1(i+1) closures are woven
            # in at evenly spaced stream positions.
            st["tc"] = tc
            st["fill"] = fill
            if only is None:
                _emit_s1_weights(st, 0)
                for n4 in range(N4):
                    _emit_s1_n4(st, 0, n4)
                for i in range(reps):
                    fillers = []
                    if i + 1 < reps:
                        fillers.append(lambda i=i: _emit_s1_weights(st, i + 1))
                        for n4 in range(N4):
                            fillers.extend(_s1_n4_closures(st, i + 1, n4))
                    if i >= 1:
                        fillers.append(lambda i=i: _emit_s3_weights(st, i - 1))
                        for b in range(32):
                            fillers.append(
                                lambda i=i, b=b: _emit_s3_block(st, i - 1, b))
                    _emit_s2_rep(st, i, fillers)
                _emit_s3_weights(st, reps - 1)
                for b in range(32):
                    _emit_s3_block(st, reps - 1, b)
            elif only == "s1":
                for i in range(reps):
                    _emit_s1_weights(st, i)
                    for n4 in range(N4):
                        _emit_s1_n4(st, i, n4)
            elif only == "s2":
                _emit_s1_weights(st, 0)
                for n4 in range(N4):
                    _emit_s1_n4(st, 0, n4)
                for i in range(reps):
                    _emit_s2_rep(st, i, [], s2par=0)
                _emit_s3_weights(st, reps - 1)
                for b in range(32):
                    _emit_s3_block(st, reps - 1, b)
            elif only == "s3":
                _emit_s1_weights(st, 0)
                for n4 in range(N4):
                    _emit_s1_n4(st, 0, n4)
                _emit_s2_rep(st, 0, [])
                for i in range(reps):
                    _emit_s3_weights(st, i)
                    for b in range(32):
                        _emit_s3_block(st, i, b, s3par=0)

    nc.compile()
    return nc


def _emit_s1_weights(st, rep):
    """w_qkv: v columns f32 (direct DMA), q/k columns fp8 DoubleRow pairs."""
    nc = st["nc"]
    wqv = [
        st["wqv_pool"].tile([P, GI], FP, tag=f"wqv{c}", name=f"wqv{c}_r{rep}")
        for c in range(NC8)
    ]
    for c in range(NC8):
        nc.sync.dma_start(
            out=_mm_cast(wqv[c]),
            in_=_mm_cast(st["wqkvT"][c * P : (c + 1) * P, 2 * GI : 3 * GI]),
        )
    st["wqv_sb"][rep] = wqv
    if S1Q_FP8:
        wq8 = [
            st["wq_pool"].tile([P, 2, 2 * GI], F8, tag=f"wq{cp}", name=f"wq{cp}_r{rep}")
            for cp in range(NCP)
        ]
        for c in range(NC8):
            wf = st["wqf_pool"].tile([P, 2 * GI], FP, tag="wqf")
            nc.sync.dma_start(out=wf, in_=st["wqkvT"][c * P : (c + 1) * P, 0 : 2 * GI])
            nc.vector.tensor_copy(wq8[c // 2][:, c % 2, :], wf)
        st["wq_sb"][rep] = wq8
    else:
        wqk = [
            st["wq_pool"].tile([P, 2 * GI], FP, tag=f"wq{c}", name=f"wq{c}_r{rep}")
            for c in range(NC8)
        ]
        for c in range(NC8):
            nc.sync.dma_start(
                out=_mm_cast(wqk[c]),
                in_=_mm_cast(st["wqkvT"][c * P : (c + 1) * P, 0 : 2 * GI]),
            )
        st["wq_sb"][rep] = wqk


def _s1_n4_closures(st, rep, n4):
    """Stage 1 chunk as a list of small closures (one PSUM chain each) so
    the stage-2 stream interleaves between them and the psA->DVE->free
    roundtrip latency never head-of-line blocks the PE queue."""
    nc = st["nc"]
    nsl = slice(n4 * 512, (n4 + 1) * 512)
    ctx = {}

    def load_x():
        xs = []
        for c in range(NC8):
            xt = st["x_pool"].tile([P, 512], FP, tag="xs")
            nc.sync.dma_start(
                out=_mm_cast(xt), in_=_mm_cast(st["xT"][c * P : (c + 1) * P, nsl])
            )
            xs.append(xt)
        ctx["xs"] = xs
        if S1Q_FP8:
            x8s = []
            for cp in range(NCP):
                x8 = st["x8_pool"].tile([P, 2, 512], F8, tag="x8s")
                nc.vector.tensor_copy(x8[:, 0, :], xs[2 * cp])
                nc.vector.tensor_copy(x8[:, 1, :], xs[2 * cp + 1])
                x8s.append(x8)
            ctx["x8s"] = x8s

    def v_chain(ms):
        xs = ctx["xs"]
        wqv = st["wqv_sb"][rep]
        mt = n4 * 4 + ms
        ps = st["psA"].tile([P, 512], FP, tag="psA")
        for c in range(NC8):
            nc.tensor.matmul(
                ps,
                lhsT=_mm_cast(xs[c][:, ms * P : (ms + 1) * P]),
                rhs=_mm_cast(wqv[c]),
                start=(c == 0),
                stop=(c == NC8 - 1),
            )
        if PV_FP8:
            v8p = st["v8p"][rep % 2]
            vv = v8p[mt // 2][:, mt % 2, 0:VROW].rearrange("p (h w) -> p h w", w=VW)
        else:
            vv = st["v8p"][rep % 2][mt].rearrange("p (h w) -> p h w", w=VW)
        nc.vector.tensor_add(
            vv[:, :, 0:HD],
            ps.rearrange("p (h w) -> p h w", w=HD),
            st["bv_bc"].rearrange("p (h w) -> p h w", w=HD),
        )

    def qk_chain(jt):
        q8h, k8h = st["q8h"][rep % 2], st["k8h"][rep % 2]
        wq_sb = st["wq_sb"][rep]
        ps = st["psA"].tile([P, 512], FP, tag="psA")
        if S1Q_FP8:
            for cp in range(NCP):
                nc.tensor.matmul(
                    ps,
                    lhsT=wq_sb[cp][:, :, jt * P : (jt + 1) * P],
                    rhs=ctx["x8s"][cp],
                    start=(cp == 0),
                    stop=(cp == NCP - 1),
                    perf_mode=DR,
                )
        else:
            for c in range(NC8):
                nc.tensor.matmul(
                    ps,
                    lhsT=_mm_cast(wq_sb[c][:, jt * P : (jt + 1) * P]),
                    rhs=_mm_cast(ctx["xs"][c]),
                    start=(c == 0),
                    stop=(c == NC8 - 1),
                )
        t = jt % 4
        dst = q8h if jt < 4 else k8h
        nc.vector.tensor_scalar_add(
            dst[2 * t][0:HD, nsl], ps[0:HD, :], st["bqk_sb"][0:HD, jt : jt + 1]
        )
        nc.vector.tensor_scalar_add(
            dst[2 * t + 1][HD:P, nsl], ps[HD:P, :], st["bqk_sb"][HD:P, jt : jt + 1]
        )

    out = [load_x]
    for ms in range(4):
        out.append(lambda ms=ms: v_chain(ms))
    for jt in (0, 4, 1, 5, 2, 6, 3, 7):
        out.append(lambda jt=jt: qk_chain(jt))
    return out


def _emit_s1_n4(st, rep, n4):
    for f in _s1_n4_closures(st, rep, n4):
        f()


def _emit_s2_rep(st, rep, fillers, s2par=None):
    """Stage 2 for one rep as a single flattened stream.

    256 mt-pair steps (8 heads x 4 q-slices x 8 pairs).  Each step: two
    512-wide QK matmuls into one [128, 2, 512] PSUM tile (2 banks), one
    [128, 1024] exp producing the fp8 probs pair tile, a DoubleRow PV
    lagged by PVLAG pairs, and deferred per-block division.  Fillers
    (stage1 of rep+1, stage3 of rep-1) are spread evenly between steps at
    low priority so cross-engine semaphore latency (~2us/hop on this HW)
    never head-of-line blocks the PE queue.
    """
    nc = st["nc"]
    tc = st["tc"]
    par = rep % 2 if s2par is None else s2par
    v8p = st["v8p"][par]
    PVLAG = 2

    po_of = {}
    pvq = []          # pending (j, pair, ptp)
    fillers = list(fillers)
    n_steps = 32 * NTP
    fill_every = max(1, n_steps // max(1, len(fillers) + 1))

    def emit_pv(j, pair, ptp):
        h = j // NQS
        nc.tensor.matmul(
            po_of[j][0:VW, :],
            lhsT=v8p[pair][:, :, h * VW : (h + 1) * VW],
            rhs=ptp,
            start=(pair == 0),
            stop=(pair == NTP - 1),
            perf_mode=DR,
        )
        if pair == NTP - 1:
            emit_division(j)

    def emit_division(j):
        h, qs = j // NQS, j % NQS
        po = po_of.pop(j)
        nsl = slice(qs * SW, (qs + 1) * SW)
        zr = st["zr_pool"].tile([1, SW], FP, tag="zr")
        nc.vector.reciprocal(zr, po[HD : HD + 1, :])
        zb = st["z_pool"].tile([HD, SW], FP, tag="zb_sb")
        nc.gpsimd.partition_broadcast(zb, zr)
        qt, prow = h // 2, (h % 2) * HD
        nc.vector.tensor_mul(
            st["cat_sb"][rep % 2][qt][prow : prow + HD, nsl], po[0:HD, :], zb
        )

    t = 0
    for j in range(32):
        h, qs = j // NQS, j % NQS
        qa = st["q8h"][par][h]
        ka = st["k8h"][par][h]
        nsl = slice(qs * SW, (qs + 1) * SW)
        po_of[j] = st["pso"].tile([P, SW], FP, tag="po", name=f"po{h}_{qs}_r{rep}")
        for pair in range(NTP):
            ps = st["ps2"].tile([P, 2, SW], FP, tag="ps_s")
            for half in range(2):
                mt = 2 * pair + half
                nc.tensor.matmul(
                    ps[:, half, :],
                    lhsT=ka[:, mt * P : (mt + 1) * P],
                    rhs=qa[:, nsl],
                    start=True,
                    stop=True,
                )
            ptp = st["probs_pool"].tile([P, 2, SW], F8, tag="pt")
            nc.scalar.activation(
                ptp, ps, mybir.ActivationFunctionType.Exp, scale=SCALE
            )
            pvq.append((j, pair, ptp))
            if len(pvq) > PVLAG:
                emit_pv(*pvq.pop(0))
            t += 1
            if fillers and t % fill_every == 0:
                f = fillers.pop(0)
                with tc.high_priority(offset=-st["fill"]):
                    f()
    while pvq:
        emit_pv(*pvq.pop(0))
    while fillers:
        f = fillers.pop(0)
        with tc.high_priority(offset=-st["fill"]):
            f()


def _emit_s3_weights(st, rep):
    nc = st["nc"]
    wp_sb = [
        st["wpb_pool"].tile([P, DIM], BF, tag=f"wpb{i}", name=f"wp{i}_r{rep}")
        for i in range(4)
    ]
    for i in range(4):
        wf = st["wpf_pool"].tile([P, DIM], FP, tag="wpf")
        nc.sync.dma_start(out=wf, in_=st["wpT"][i * P : (i + 1) * P, :])
        nc.vector.tensor_copy(wp_sb[i], wf)
    st["wp_sb"][rep] = wp_sb


def _emit_s3_block(st, rep, b, s3par=None):
    """Stage 3 block: one (nt, o2) output projection chunk (bf16 -> bf16)."""
    nc = st["nc"]
    nt, o2 = b // 2, b % 2
    wp_sb = st["wp_sb"][rep]
    cat_sb = st["cat_sb"][rep % 2 if s3par is None else s3par]
    osl = slice(o2 * 512, (o2 + 1) * 512)
    ps = st["psA"].tile([P, 512], FP, tag="psA")
    for ic in range(4):
        nc.tensor.matmul(
            ps,
            lhsT=cat_sb[ic][:, nt * P : (nt + 1) * P],
            rhs=wp_sb[ic][:, osl],
            start=(ic == 0),
            stop=(ic == 3),
        )
    ot = st["outp"].tile([P, 512], BF, tag="ot")
    nc.vector.tensor_add(ot, ps, st["bp_bc"][:, osl])
    nc.sync.dma_start(out=st["part"][nt * P : (nt + 1) * P, osl], in_=ot)


_NC = None
_EXEC_CACHE = {}


def _get_nc():
    global _NC
    if _NC is None:
        _NC = build_nc()
    return _NC


def _make_in_maps(x, w_qkv, b_qkv, w_proj, b_proj):
    x = np.asarray(x, np.float32)
    w_qkv = np.asarray(w_qkv, np.float32)
    b_qkv = np.asarray(b_qkv, np.float32)
    w_proj = np.asarray(w_proj, np.float32)
    b_proj = np.asarray(b_proj, np.float32)
    in_maps = []
    for c in range(8):
        b, g = c // 2, c % 2
        hsl = slice(g * GI, (g + 1) * GI)
        wq = w_qkv[0 * DIM + g * GI : 0 * DIM + (g + 1) * GI]
        wk = w_qkv[1 * DIM + g * GI : 1 * DIM + (g + 1) * GI]
        wv = w_qkv[2 * DIM + g * GI : 2 * DIM + (g + 1) * GI]
        wqkvT = np.ascontiguousarray(np.concatenate([wq, wk, wv], 0).T)
        bq = b_qkv[0 * DIM + g * GI : 0 * DIM + (g + 1) * GI]
        bk = b_qkv[1 * DIM + g * GI : 1 * DIM + (g + 1) * GI]
        bv_ = b_qkv[2 * DIM + g * GI : 2 * DIM + (g + 1) * GI]
        in_maps.append(
            {
                "xT": np.ascontiguousarray(x[b].T),
                "wqkvT": wqkvT,
                "bqk": np.ascontiguousarray(np.concatenate([bq, bk])),
                "bv": np.ascontiguousarray(bv_),
                "wpT": np.ascontiguousarray(w_proj[:, hsl].T),
                "bph": np.ascontiguousarray(b_proj * 0.5),
            }
        )
    return in_maps


def _nc_io(nc):
    """(in_names, out_names, out_avals) from the compiled module."""
    import jax

    in_names, out_names, out_avals = [], [], []
    for alloc in nc.m.functions[0].allocations:
        if not isinstance(alloc, mybir.MemoryLocationSet):
            continue
        name = alloc.memorylocations[0].name
        if alloc.kind == "ExternalInput":
            if nc.partition_id_tensor and name == nc.partition_id_tensor.name:
                continue
            in_names.append(name)
        elif alloc.kind == "ExternalOutput":
            out_names.append(name)
            out_avals.append(
                jax.core.ShapedArray(tuple(alloc.tensor_shape), mybir.dt.np(alloc.dtype))
            )
    return in_names, out_names, out_avals


def _make_exec(nc):
    """Build (and cache) the 8-core sharded jit callable for `nc`."""
    if id(nc) in _EXEC_CACHE:
        return _EXEC_CACHE[id(nc)]

    import jax
    from jax.sharding import Mesh, PartitionSpec
    from jax.experimental.shard_map import shard_map
    from concourse import bass2jax

    bass2jax.install_neuronx_cc_hook()
    in_names, out_names, out_avals = _nc_io(nc)
    n_params = len(in_names)
    partition_name = nc.partition_id_tensor.name if nc.partition_id_tensor else None
    all_in_names = tuple(in_names) + tuple(out_names)
    if partition_name is not None:
        all_in_names = all_in_names + (partition_name,)

    def _exec(*args):
        operands = list(args)
        if partition_name is not None:
            operands.append(bass2jax.partition_id_tensor())
        outs = bass2jax._bass_exec_p.bind(
            *operands,
            out_avals=tuple(out_avals),
            in_names=all_in_names,
            out_names=tuple(out_names),
            lowering_input_output_aliases=(),
            sim_require_finite=True,
            sim_require_nnan=True,
            nc=nc,
        )
        return tuple(outs)

    mesh = Mesh(np.asarray(jax.devices()[:8]), ("core",))
    sharded = jax.jit(
        shard_map(
            _exec,
            mesh=mesh,
            in_specs=(PartitionSpec("core"),) * (n_params + len(out_names)),
            out_specs=(PartitionSpec("core"),) * len(out_names),
            check_rep=False,
        )
    )
    entry = (sharded, mesh, in_names, out_names, out_avals)
    _EXEC_CACHE[id(nc)] = entry
    return entry


def _device_inputs(nc, in_maps):
    """Concatenate per-core inputs and place them sharded across the mesh."""
    import jax
    from jax.sharding import NamedSharding, PartitionSpec

    sharded, mesh, in_names, out_names, out_avals = _make_exec(nc)
    per_core = [[np.asarray(m[n]) for n in in_names] for m in in_maps]
    concat_in = [
        np.concatenate([per_core[c][i] for c in range(8)], 0)
        for i in range(len(in_names))
    ]
    concat_in += [
        np.zeros((8 * av.shape[0], *av.shape[1:]), av.dtype) for av in out_avals
    ]
    spec = NamedSharding(mesh, PartitionSpec("core"))
    return [jax.device_put(a, spec) for a in concat_in]


def _exec_out_to_full(outs):
    """Assemble the full [B, N, DIM] output from the concatenated parts."""
    parts_cat = np.asarray(outs[0]).astype(np.float32).reshape(8, N, DIM)
    out = np.empty((B, N, DIM), np.float32)
    for b in range(B):
        out[b] = parts_cat[2 * b] + parts_cat[2 * b + 1]
    return out


def kernel(x, w_qkv, b_qkv, w_proj, b_proj):
    import jax

    nc = _get_nc()
    in_maps = _make_in_maps(x, w_qkv, b_qkv, w_proj, b_proj)
    sharded, mesh, in_names, out_names, out_avals = _make_exec(nc)
    dev_in = _device_inputs(nc, in_maps)
    outs = sharded(*dev_in)
    jax.block_until_ready(outs)
    return _exec_out_to_full(outs)


def bench(x, w_qkv, b_qkv, w_proj, b_proj, rep_counts=(1, 65), batch=10, rounds=12):
    """Returns (out, per_iter_exec_ns, info); marginal cost of in-NEFF reps."""
    import jax

    in_maps = _make_in_maps(x, w_qkv, b_qkv, w_proj, b_proj)

    ncs = [(_get_nc() if k == 1 else build_nc(reps=k)) for k in rep_counts]
    fns = [_make_exec(nc)[0] for nc in ncs]
    dev_in = _device_inputs(ncs[0], in_maps)

    outs = fns[0](*dev_in)
    jax.block_until_ready(outs)  # compile + warm
    for fn in fns[1:]:
        jax.block_until_ready(fn(*dev_in))

    ts = [[] for _ in fns]
    for _ in range(rounds):
        for i, fn in enumerate(fns):
            t0 = time.perf_counter()
            os_ = [fn(*dev_in) for _ in range(batch)]
            jax.block_until_ready(os_)
            ts[i].append((time.perf_counter() - t0) / batch)

    meds = [float(np.median(t)) for t in ts]
    per_iter = (meds[-1] - meds[0]) / (rep_counts[-1] - rep_counts[0])
    out = _exec_out_to_full(outs)
    info = {
        "rep_counts": list(rep_counts),
        "batch": batch,
        "rounds": rounds,
        "med_percall_ms": [round(m * 1e3, 3) for m in meds],
    }
    return out, int(per_iter * 1e9), info
